# revision 1
# baseline (speedup 1.0000x reference)
"""Trainium2 Bass kernel: conv2d(3x3, VALID) + bias -> channel-min -> tanh(tanh).

Full inputs in, full output out. Data-parallel over batch across 8 NeuronCores.

Per-core compute scheme (weight-stationary conv as matmul):
  - Output rows are processed in (delta, t) pairs: h' = 2*t + delta, delta in {0,1}.
  - Matmul M-dim packs (delta, oc): M = 2*64 = 128 output partitions.
  - Contraction K packs (khe, ic) where khe = delta + kh in [0,4): K = 4*16 = 64.
  - 3 PSUM-accumulated matmuls per tile, one per kw (kw enters as a uniform
    free-dim offset into a row-shifted image copy).
  - Host pre-builds 4 row-shifted copies of the image (khe shifts) in bf16,
    so the rhs AP for each matmul is a plain strided read.
  - Two batches are processed concurrently on disjoint PE row halves
    (partitions 0-63 / 64-127) via base_partition=64 row tiling.
  - PSUM [128=(delta,oc), N] is evacuated to SBUF bf16 by ScalarE with the
    conv bias fused in (activation Identity + per-partition bias).
  - DMA xbar transpose flips [ch, px] -> [px, ch] so the channel-min becomes
    a free-dim reduction tree on VectorE (bf16 2x mode).
  - Double tanh on ScalarE, DMA out.
"""

import os
import sys

for _p in ("/opt/trn_rl_repo", "/root/.axon_site/_ro/trn_rl_repo"):
    if os.path.isdir(_p) and _p not in sys.path:
        sys.path.insert(0, _p)

import numpy as np
import ml_dtypes

import concourse.bass as bass
import concourse.bacc as bacc
import concourse.tile as tile
from concourse import mybir
from concourse.bass_utils import run_bass_kernel_spmd

N_CORES = 8
B, IC, H, W = 128, 16, 128, 128
OC, KSZ = 64, 3
HO, WO = H - KSZ + 1, W - KSZ + 1  # 126, 126
B_LOC = B // N_CORES  # 16
PAIRS = B_LOC // 2  # 8
T = HO // 2  # 63 row-pairs per image (h' = 2t + delta)
FLAT = H * W  # 16384

BF16 = mybir.dt.bfloat16
F32 = mybir.dt.float32

# t-groups of up to 4 row-pairs -> matmul N = cnt*128
GROUPS = [(t0, min(4, T - t0)) for t0 in range(0, T, 4)]  # 16 groups, last cnt=3
# blocks of groups sharing one transpose: blk0 = t 0..31 (8 groups),
# blk1 = t 32..62 (8 groups, 31 rows)
BLOCKS = [GROUPS[:8], GROUPS[8:]]


def _build_program():
    nc = bacc.Bacc(None)
    xr_hbm = nc.declare_dram_parameter(
        "xrep", [PAIRS, 128, FLAT], BF16, isOutput=False
    )
    w_hbm = nc.declare_dram_parameter("wts", [128, 3 * 128], BF16, isOutput=False)
    b_hbm = nc.declare_dram_parameter("bias", [128, 1], F32, isOutput=False)
    y_hbm = nc.declare_dram_parameter("y", [B_LOC, HO, WO], F32, isOutput=True)

    with tile.TileContext(nc) as tc:
        with (
            tc.tile_pool(name="const", bufs=1) as const,
            tc.tile_pool(name="xrp", bufs=2) as xrp,
            tc.tile_pool(name="psum", bufs=6, space="PSUM") as psump,
            tc.tile_pool(name="evac", bufs=3) as evacp,
            tc.tile_pool(name="tpose", bufs=3) as tposep,
            tc.tile_pool(name="tree", bufs=3) as treep,
            tc.tile_pool(name="outp", bufs=3) as outp,
        ):
            w_sb = const.tile([128, 3 * 128], BF16)
            b_sb = const.tile([128, 1], F32)
            nc.sync.dma_start(w_sb[:], w_hbm[:])
            nc.sync.dma_start(b_sb[:], b_hbm[:])

            for pair in range(PAIRS):
                xr = xrp.tile([128, FLAT], BF16)
                nc.sync.dma_start(xr[:], xr_hbm[pair])
                # view: free dim as 64 double-rows of 256 (row r=2t at offset t*256)
                xrv = xr.rearrange("p (r q) -> p r q", q=2 * W)
                for half in range(2):
                    bidx = pair * 2 + half
                    pl, ph = 64 * half, 64 * half + 64
                    out_sb = outp.tile([128, HO], F32)
                    for blk_i, blk in enumerate(BLOCKS):
                        nt = sum(c for _, c in blk)  # 32 or 31
                        conv_sb = evacp.tile([128, 32 * 128], BF16)
                        off = 0
                        for t0, cnt in blk:
                            n = cnt * 128
                            ps = psump.tile([128, 512], F32)
                            for kw in range(3):
                                nc.tensor.matmul(
                                    ps[:, :n],
                                    w_sb[pl:ph, kw * 128 : (kw + 1) * 128],
                                    xrv[pl:ph, t0 : t0 + cnt, kw : kw + 128],
                                    start=(kw == 0),
                                    stop=(kw == 2),
                                )
                            nc.scalar.activation(
                                conv_sb[:, off : off + n],
                                ps[:, :n],
                                mybir.ActivationFunctionType.Identity,
                                bias=b_sb[:, 0:1],
                            )
                            off += n
                        # transpose [128=(d,oc), nt*128=(t,w')] -> [w', t, (d,oc)]
                        tp = tposep.tile([128, 32 * 128], BF16)
                        nc.sync.dma_start_transpose(
                            tp.rearrange("p (j c) -> p j c", c=128)[:, :nt, :],
                            conv_sb[:, : nt * 128],
                        )
                        # min-tree over oc (free dim), keeping (t, delta)
                        cur = tp
                        width = 64
                        while width > 1:
                            w2 = width // 2
                            nxt = treep.tile([128, 32 * 2 * w2], BF16, tag=f"tl{w2}")
                            cv = cur.rearrange("p (j d c) -> p j d c", d=2, c=width)
                            nv = nxt.rearrange("p (j d c) -> p j d c", d=2, c=w2)
                            nc.vector.tensor_tensor(
                                nv[:, :nt, :, :],
                                cv[:, :nt, :, 0:w2],
                                cv[:, :nt, :, w2:width],
                                mybir.AluOpType.min,
                            )
                            cur = nxt
                            width = w2
                        # double tanh
                        th = treep.tile([128, 32 * 2], F32, tag="th")
                        nc.scalar.activation(
                            th[:, : nt * 2],
                            cur[:, : nt * 2],
                            mybir.ActivationFunctionType.Tanh,
                        )
                        nc.scalar.activation(
                            out_sb[:, blk_i * 64 : blk_i * 64 + nt * 2],
                            th[:, : nt * 2],
                            mybir.ActivationFunctionType.Tanh,
                        )
                    # out_sb[w', (t, d)] -> y[bidx, 2t+d, w']
                    nc.sync.dma_start(
                        y_hbm[bidx].rearrange("(t d) w -> w t d", d=2),
                        out_sb[0:WO, :].rearrange("w (t d) -> w t d", d=2),
                    )
    nc.finalize()
    return nc


_NC_CACHE = None


def _get_program():
    global _NC_CACHE
    if _NC_CACHE is None:
        _NC_CACHE = _build_program()
    return _NC_CACHE


def _host_prep(x, conv_weight, conv_bias):
    # x: [B, IC, H, W] f32
    # xrep[b, khe, ic, r, :] = x[b, ic, r+khe, :]  (zero past the end)
    xb = x.astype(ml_dtypes.bfloat16)
    xrep = np.zeros((B, 4, IC, H, W), dtype=ml_dtypes.bfloat16)
    for khe in range(4):
        xrep[:, khe, :, : H - khe, :] = xb[:, :, khe:, :]
    # per-core: [B_LOC, 4*IC, FLAT] -> pairs [PAIRS, 128, FLAT]
    xrep = xrep.reshape(B, 4 * IC, FLAT)

    # weights: Wl[p=(khe*16+ic), kw, m=(delta*64+oc)] = w[oc, ic, khe-delta, kw]
    wl = np.zeros((64, 3, 128), dtype=np.float32)
    for khe in range(4):
        for dlt in range(2):
            kh = khe - dlt
            if 0 <= kh < KSZ:
                # conv_weight[:, :, kh, :] : [OC, IC, KW] -> [ic, kw, oc]
                wl[khe * 16 : khe * 16 + 16, :, dlt * 64 : dlt * 64 + 64] = (
                    conv_weight[:, :, kh, :].transpose(1, 2, 0)
                )
    wts = np.concatenate([wl, wl], axis=0).reshape(128, 3 * 128)
    wts = wts.astype(ml_dtypes.bfloat16)

    biasarr = np.tile(conv_bias.astype(np.float32), 2).reshape(128, 1)
    return xrep, wts, biasarr


def kernel(x, conv_weight, conv_bias):
    x = np.asarray(x, dtype=np.float32)
    conv_weight = np.asarray(conv_weight, dtype=np.float32)
    conv_bias = np.asarray(conv_bias, dtype=np.float32)

    xrep, wts, biasarr = _host_prep(x, conv_weight, conv_bias)

    in_maps = []
    for c in range(N_CORES):
        xc = xrep[c * B_LOC : (c + 1) * B_LOC]  # [B_LOC, 64, FLAT]
        xc = np.ascontiguousarray(xc).reshape(PAIRS, 128, FLAT)
        in_maps.append({"xrep": xc, "wts": wts, "bias": biasarr})

    nc = _get_program()
    res = run_bass_kernel_spmd(nc, in_maps, list(range(N_CORES)))
    y = np.concatenate([res.results[c]["y"] for c in range(N_CORES)], axis=0)
    return y.reshape(B, 1, HO, WO).astype(np.float32)


# revision 5
# speedup vs baseline: 1.0828x; 1.0828x over previous
"""Trainium2 Bass kernel: conv2d(3x3, VALID) + bias -> channel-min -> tanh(tanh).

Full inputs in, full output out. Data-parallel over batch across 8 NeuronCores.

Per-core compute scheme (weight-stationary conv as matmul):
  - Output rows are processed in (delta, t) pairs: h' = 2*t + delta, delta in {0,1}.
  - Matmul M-dim packs (delta, oc): M = 2*64 = 128 output partitions.
  - Contraction K packs (khe, ic) where khe = delta + kh in [0,4): K = 4*16 = 64.
  - 3 PSUM-accumulated matmuls per tile, one per kw (kw enters as a uniform
    free-dim offset into a row-shifted image copy).
  - Host pre-builds 4 row-shifted copies of the image (khe shifts) in bf16,
    so the rhs AP for each matmul is a plain strided read.
  - Two batches are processed concurrently on disjoint PE row halves
    (partitions 0-63 / 64-127) via base_partition=64 row tiling.
  - PSUM [128=(delta,oc), N] is evacuated to SBUF bf16 by ScalarE with the
    conv bias fused in (activation Identity + per-partition bias).
  - DMA xbar transpose flips [ch, px] -> [px, ch] so the channel-min becomes
    a free-dim reduction tree on VectorE (bf16 2x mode).
  - Double tanh on ScalarE, DMA out.
"""

import os
import sys

for _p in ("/opt/trn_rl_repo", "/root/.axon_site/_ro/trn_rl_repo"):
    if os.path.isdir(_p) and _p not in sys.path:
        sys.path.insert(0, _p)

import numpy as np
import ml_dtypes

import concourse.bass as bass
import concourse.bacc as bacc
import concourse.tile as tile
from concourse import mybir
from concourse.bass_utils import run_bass_kernel_spmd

N_CORES = 8
B, IC, H, W = 128, 16, 128, 128
OC, KSZ = 64, 3
HO, WO = H - KSZ + 1, W - KSZ + 1  # 126, 126
B_LOC = B // N_CORES  # 16
PAIRS = B_LOC // 2  # 8
T = HO // 2  # 63 row-pairs per image (h' = 2t + delta)
FLAT = H * W  # 16384

BF16 = mybir.dt.bfloat16
F32 = mybir.dt.float32

# t-groups of up to 4 row-pairs -> matmul N = cnt*128
GROUPS = [(t0, min(4, T - t0)) for t0 in range(0, T, 4)]  # 16 groups, last cnt=3
# blocks of groups sharing one transpose: blk0 = t 0..31 (8 groups),
# blk1 = t 32..62 (8 groups, 31 rows)
BLOCKS = [GROUPS[:8], GROUPS[8:]]


def _build_program():
    nc = bacc.Bacc(None)
    xr_hbm = nc.declare_dram_parameter(
        "xrep", [PAIRS, 128, FLAT], BF16, isOutput=False
    )
    w_hbm = nc.declare_dram_parameter("wts", [128, 3 * 128], BF16, isOutput=False)
    b_hbm = nc.declare_dram_parameter("bias", [128, 1], F32, isOutput=False)
    y_hbm = nc.declare_dram_parameter("y", [B_LOC, HO, WO], F32, isOutput=True)

    with tile.TileContext(nc) as tc:
        with (
            tc.tile_pool(name="const", bufs=1) as const,
            tc.tile_pool(name="xrp", bufs=2) as xrp,
            tc.tile_pool(name="psum", bufs=8, space="PSUM") as psump,
            tc.tile_pool(name="evac", bufs=4) as evacp,
            tc.tile_pool(name="tpose", bufs=4) as tposep,
            tc.tile_pool(name="tree", bufs=4) as treep,
            tc.tile_pool(name="outp", bufs=3) as outp,
        ):
            w_sb = const.tile([128, 3 * 128], BF16)
            b_sb = const.tile([128, 1], F32)
            nc.sync.dma_start(w_sb[:], w_hbm[:])
            nc.sync.dma_start(b_sb[:], b_hbm[:])

            tpose_cnt = 0
            for pair in range(PAIRS):
                xr = xrp.tile([128, FLAT], BF16)
                # SWDGE ring: doesn't contend with the HWDGE transpose rings
                nc.sync.dma_start(xr[:], xr_hbm[pair])
                # view: free dim as 64 double-rows of 256 (row r=2t at offset t*256)
                xrv = xr.rearrange("p (r q) -> p r q", q=2 * W)
                out_sbs = [outp.tile([128, HO], F32, tag=f"out{h}", name=f"out{h}") for h in range(2)]
                for blk_i, blk in enumerate(BLOCKS):
                    nt = sum(c for _, c in blk)  # 32 or 31
                    conv_sbs = [
                        evacp.tile([128, 32 * 128], BF16, tag=f"cv{h}", name=f"cv{h}")
                        for h in range(2)
                    ]
                    off = 0
                    for gi, (t0, cnt) in enumerate(blk):
                        n = cnt * 128
                        for half in range(2):
                            pl, ph = 64 * half, 64 * half + 64
                            ps = psump.tile([128, 512], F32)
                            for kw in range(3):
                                nc.tensor.matmul(
                                    ps[:, :n],
                                    w_sb[pl:ph, kw * 128 : (kw + 1) * 128],
                                    xrv[pl:ph, t0 : t0 + cnt, kw : kw + 128],
                                    start=(kw == 0),
                                    stop=(kw == 2),
                                )
                            # evacuate PSUM -> SBUF bf16 with bias add;
                            # split between ScalarE and VectorE by load balance
                            dst = conv_sbs[half][:, off : off + n]
                            if gi % 4 < 3:
                                nc.scalar.activation(
                                    dst,
                                    ps[:, :n],
                                    mybir.ActivationFunctionType.Identity,
                                    bias=b_sb[:, 0:1],
                                )
                            else:
                                nc.vector.tensor_scalar(
                                    dst,
                                    ps[:, :n],
                                    b_sb[:, 0:1],
                                    None,
                                    mybir.AluOpType.add,
                                )
                        off += n
                    for half in range(2):
                        # transpose [128=(d,oc), nt*128=(t,w')] -> [w', t, (d,oc)]
                        tp = tposep.tile([128, 32 * 128], BF16)
                        teng = nc.sync
                        tpose_cnt += 1
                        teng.dma_start_transpose(
                            tp.rearrange("p (j c) -> p j c", c=128)[:, :nt, :],
                            conv_sbs[half][:, : nt * 128],
                        )
                        # min-tree over oc (free dim), keeping (t, delta)
                        cur = tp
                        width = 64
                        while width > 1:
                            w2 = width // 2
                            nxt = treep.tile([128, 32 * 2 * w2], BF16, tag=f"tl{w2}")
                            cv = cur.rearrange("p (j d c) -> p j d c", d=2, c=width)
                            nv = nxt.rearrange("p (j d c) -> p j d c", d=2, c=w2)
                            nc.vector.tensor_tensor(
                                nv[:, :nt, :, :],
                                cv[:, :nt, :, 0:w2],
                                cv[:, :nt, :, w2:width],
                                mybir.AluOpType.min,
                            )
                            cur = nxt
                            width = w2
                        # double tanh
                        th = treep.tile([128, 32 * 2], F32, tag="th")
                        nc.scalar.activation(
                            th[:, : nt * 2],
                            cur[:, : nt * 2],
                            mybir.ActivationFunctionType.Tanh,
                        )
                        nc.scalar.activation(
                            out_sbs[half][:, blk_i * 64 : blk_i * 64 + nt * 2],
                            th[:, : nt * 2],
                            mybir.ActivationFunctionType.Tanh,
                        )
                for half in range(2):
                    # out_sb[w', (t, d)] -> y[bidx, 2t+d, w']
                    nc.sync.dma_start(
                        y_hbm[pair * 2 + half].rearrange("(t d) w -> w t d", d=2),
                        out_sbs[half][0:WO, :].rearrange("w (t d) -> w t d", d=2),
                    )
    nc.finalize()
    return nc


_NC_CACHE = None


def _get_program():
    global _NC_CACHE
    if _NC_CACHE is None:
        _NC_CACHE = _build_program()
    return _NC_CACHE


def _host_prep(x, conv_weight, conv_bias):
    # x: [B, IC, H, W] f32
    # xrep[b, khe, ic, r, :] = x[b, ic, r+khe, :]  (zero past the end)
    xb = x.astype(ml_dtypes.bfloat16)
    xrep = np.zeros((B, 4, IC, H, W), dtype=ml_dtypes.bfloat16)
    for khe in range(4):
        xrep[:, khe, :, : H - khe, :] = xb[:, :, khe:, :]
    # per-core: [B_LOC, 4*IC, FLAT] -> pairs [PAIRS, 128, FLAT]
    xrep = xrep.reshape(B, 4 * IC, FLAT)

    # weights: Wl[p=(khe*16+ic), kw, m=(delta*64+oc)] = w[oc, ic, khe-delta, kw]
    wl = np.zeros((64, 3, 128), dtype=np.float32)
    for khe in range(4):
        for dlt in range(2):
            kh = khe - dlt
            if 0 <= kh < KSZ:
                # conv_weight[:, :, kh, :] : [OC, IC, KW] -> [ic, kw, oc]
                wl[khe * 16 : khe * 16 + 16, :, dlt * 64 : dlt * 64 + 64] = (
                    conv_weight[:, :, kh, :].transpose(1, 2, 0)
                )
    wts = np.concatenate([wl, wl], axis=0).reshape(128, 3 * 128)
    wts = wts.astype(ml_dtypes.bfloat16)

    biasarr = np.tile(conv_bias.astype(np.float32), 2).reshape(128, 1)
    return xrep, wts, biasarr


def kernel(x, conv_weight, conv_bias):
    x = np.asarray(x, dtype=np.float32)
    conv_weight = np.asarray(conv_weight, dtype=np.float32)
    conv_bias = np.asarray(conv_bias, dtype=np.float32)

    xrep, wts, biasarr = _host_prep(x, conv_weight, conv_bias)

    in_maps = []
    for c in range(N_CORES):
        xc = xrep[c * B_LOC : (c + 1) * B_LOC]  # [B_LOC, 64, FLAT]
        xc = np.ascontiguousarray(xc).reshape(PAIRS, 128, FLAT)
        in_maps.append({"xrep": xc, "wts": wts, "bias": biasarr})

    nc = _get_program()
    res = run_bass_kernel_spmd(nc, in_maps, list(range(N_CORES)))
    y = np.concatenate([res.results[c]["y"] for c in range(N_CORES)], axis=0)
    return y.reshape(B, 1, HO, WO).astype(np.float32)


# revision 6
# speedup vs baseline: 2.0519x; 1.8950x over previous
"""Trainium2 Bass kernel: conv2d(3x3, VALID) + bias -> channel-min -> tanh(tanh).

Full inputs in, full output out. Data-parallel over batch across 8 NeuronCores.

Per-core compute scheme (weight-stationary conv as matmul):
  - Output rows are processed in (delta, t) pairs: h' = 2*t + delta, delta in {0,1}.
  - Matmul M-dim packs (delta, oc): M = 2*64 = 128 output partitions.
  - Contraction K packs (khe, ic) where khe = delta + kh in [0,4): K = 4*16 = 64.
  - 3 PSUM-accumulated matmuls per tile, one per kw (kw enters as a uniform
    free-dim offset into a row-shifted image copy).
  - Host pre-builds 4 row-shifted copies of the image (khe shifts) in bf16,
    so the rhs AP for each matmul is a plain strided read.
  - Two batches are processed concurrently on disjoint PE row halves
    (partitions 0-63 / 64-127) via base_partition=64 row tiling.
  - PSUM [128=(delta,oc), N] is evacuated to SBUF bf16 by ScalarE with the
    conv bias fused in (activation Identity + per-partition bias).
  - DMA xbar transpose flips [ch, px] -> [px, ch] so the channel-min becomes
    a free-dim reduction tree on VectorE (bf16 2x mode).
  - Double tanh on ScalarE, DMA out.
"""

import os
import sys

for _p in ("/opt/trn_rl_repo", "/root/.axon_site/_ro/trn_rl_repo"):
    if os.path.isdir(_p) and _p not in sys.path:
        sys.path.insert(0, _p)

import numpy as np
import ml_dtypes

import concourse.bass as bass
import concourse.bacc as bacc
import concourse.tile as tile
from concourse import mybir
from concourse.bass_utils import run_bass_kernel_spmd

N_CORES = 8
B, IC, H, W = 128, 16, 128, 128
OC, KSZ = 64, 3
HO, WO = H - KSZ + 1, W - KSZ + 1  # 126, 126
B_LOC = B // N_CORES  # 16
PAIRS = B_LOC // 2  # 8
T = HO // 2  # 63 row-pairs per image (h' = 2t + delta)
FLAT = H * W  # 16384

BF16 = mybir.dt.bfloat16
F32 = mybir.dt.float32

# t-groups of up to 4 row-pairs -> matmul N = cnt*128
GROUPS = [(t0, min(4, T - t0)) for t0 in range(0, T, 4)]  # 16 groups, last cnt=3
# blocks of groups sharing one transpose: blk0 = t 0..31 (8 groups),
# blk1 = t 32..62 (8 groups, 31 rows)
BLOCKS = [GROUPS[:8], GROUPS[8:]]


def _build_program():
    nc = bacc.Bacc(None)
    xr_hbm = nc.declare_dram_parameter(
        "xrep", [PAIRS, 128, FLAT], BF16, isOutput=False
    )
    w_hbm = nc.declare_dram_parameter("wts", [128, 3 * 128], BF16, isOutput=False)
    b_hbm = nc.declare_dram_parameter("bias", [128, 1], F32, isOutput=False)
    y_hbm = nc.declare_dram_parameter("y", [B_LOC, WO, T * 2], F32, isOutput=True)

    with tile.TileContext(nc) as tc:
        with (
            tc.tile_pool(name="const", bufs=1) as const,
            tc.tile_pool(name="xrp", bufs=2) as xrp,
            tc.tile_pool(name="psum", bufs=8, space="PSUM") as psump,
            tc.tile_pool(name="evac", bufs=4) as evacp,
            tc.tile_pool(name="tpose", bufs=4) as tposep,
            tc.tile_pool(name="tree", bufs=4) as treep,
            tc.tile_pool(name="outp", bufs=3) as outp,
        ):
            w_sb = const.tile([128, 3 * 128], BF16)
            b_sb = const.tile([128, 1], F32)
            nc.sync.dma_start(w_sb[:], w_hbm[:])
            nc.sync.dma_start(b_sb[:], b_hbm[:])

            tpose_cnt = 0
            for pair in range(PAIRS):
                xr = xrp.tile([128, FLAT], BF16)
                # SWDGE ring: doesn't contend with the HWDGE transpose rings
                nc.scalar.dma_start(xr[:], xr_hbm[pair])
                # view: free dim as 64 double-rows of 256 (row r=2t at offset t*256)
                xrv = xr.rearrange("p (r q) -> p r q", q=2 * W)
                out_sbs = [outp.tile([128, HO], F32, tag=f"out{h}", name=f"out{h}") for h in range(2)]
                for blk_i, blk in enumerate(BLOCKS):
                    nt = sum(c for _, c in blk)  # 32 or 31
                    conv_sbs = [
                        evacp.tile([128, 32 * 128], BF16, tag=f"cv{h}", name=f"cv{h}")
                        for h in range(2)
                    ]
                    off = 0
                    for gi, (t0, cnt) in enumerate(blk):
                        n = cnt * 128
                        for half in range(2):
                            pl, ph = 64 * half, 64 * half + 64
                            ps = psump.tile([128, 512], F32)
                            for kw in range(3):
                                nc.tensor.matmul(
                                    ps[:, :n],
                                    w_sb[pl:ph, kw * 128 : (kw + 1) * 128],
                                    xrv[pl:ph, t0 : t0 + cnt, kw : kw + 128],
                                    start=(kw == 0),
                                    stop=(kw == 2),
                                )
                            # evacuate PSUM -> SBUF bf16 with bias add;
                            # split between ScalarE and VectorE by load balance
                            dst = conv_sbs[half][:, off : off + n]
                            if (gi * 2 + half) % 16 < 13:
                                nc.scalar.activation(
                                    dst,
                                    ps[:, :n],
                                    mybir.ActivationFunctionType.Identity,
                                    bias=b_sb[:, 0:1],
                                )
                            else:
                                nc.vector.tensor_scalar(
                                    dst,
                                    ps[:, :n],
                                    b_sb[:, 0:1],
                                    None,
                                    mybir.AluOpType.add,
                                )
                        off += n
                    for half in range(2):
                        # transpose [128=(d,oc), nt*128=(t,w')] -> [w', t, (d,oc)]
                        tp = tposep.tile([128, 32 * 128], BF16)
                        teng = nc.sync
                        tpose_cnt += 1
                        teng.dma_start_transpose(
                            tp.rearrange("p (j c) -> p j c", c=128)[:, :nt, :],
                            conv_sbs[half][:, : nt * 128],
                        )
                        # min-tree over oc (free dim), keeping (t, delta)
                        cur = tp
                        width = 64
                        while width > 1:
                            w2 = width // 2
                            nxt = treep.tile([128, 32 * 2 * w2], BF16, tag=f"tl{w2}")
                            cv = cur.rearrange("p (j d c) -> p j d c", d=2, c=width)
                            nv = nxt.rearrange("p (j d c) -> p j d c", d=2, c=w2)
                            nc.vector.tensor_tensor(
                                nv[:, :nt, :, :],
                                cv[:, :nt, :, 0:w2],
                                cv[:, :nt, :, w2:width],
                                mybir.AluOpType.min,
                            )
                            cur = nxt
                            width = w2
                        # double tanh
                        th = treep.tile([128, 32 * 2], F32, tag="th")
                        nc.scalar.activation(
                            th[:, : nt * 2],
                            cur[:, : nt * 2],
                            mybir.ActivationFunctionType.Tanh,
                        )
                        nc.scalar.activation(
                            out_sbs[half][:, blk_i * 64 : blk_i * 64 + nt * 2],
                            th[:, : nt * 2],
                            mybir.ActivationFunctionType.Tanh,
                        )
                for half in range(2):
                    # contiguous store in permuted layout [w', (t, d)];
                    # host transposes back to [h', w']
                    nc.scalar.dma_start(
                        y_hbm[pair * 2 + half],
                        out_sbs[half][0:WO, :],
                    )
    nc.finalize()
    return nc


_NC_CACHE = None


def _get_program():
    global _NC_CACHE
    if _NC_CACHE is None:
        _NC_CACHE = _build_program()
    return _NC_CACHE


def _host_prep(x, conv_weight, conv_bias):
    # x: [B, IC, H, W] f32
    # xrep[b, khe, ic, r, :] = x[b, ic, r+khe, :]  (zero past the end)
    xb = x.astype(ml_dtypes.bfloat16)
    xrep = np.zeros((B, 4, IC, H, W), dtype=ml_dtypes.bfloat16)
    for khe in range(4):
        xrep[:, khe, :, : H - khe, :] = xb[:, :, khe:, :]
    # per-core: [B_LOC, 4*IC, FLAT] -> pairs [PAIRS, 128, FLAT]
    xrep = xrep.reshape(B, 4 * IC, FLAT)

    # weights: Wl[p=(khe*16+ic), kw, m=(delta*64+oc)] = w[oc, ic, khe-delta, kw]
    wl = np.zeros((64, 3, 128), dtype=np.float32)
    for khe in range(4):
        for dlt in range(2):
            kh = khe - dlt
            if 0 <= kh < KSZ:
                # conv_weight[:, :, kh, :] : [OC, IC, KW] -> [ic, kw, oc]
                wl[khe * 16 : khe * 16 + 16, :, dlt * 64 : dlt * 64 + 64] = (
                    conv_weight[:, :, kh, :].transpose(1, 2, 0)
                )
    wts = np.concatenate([wl, wl], axis=0).reshape(128, 3 * 128)
    wts = wts.astype(ml_dtypes.bfloat16)

    biasarr = np.tile(conv_bias.astype(np.float32), 2).reshape(128, 1)
    return xrep, wts, biasarr


def kernel(x, conv_weight, conv_bias):
    x = np.asarray(x, dtype=np.float32)
    conv_weight = np.asarray(conv_weight, dtype=np.float32)
    conv_bias = np.asarray(conv_bias, dtype=np.float32)

    xrep, wts, biasarr = _host_prep(x, conv_weight, conv_bias)

    in_maps = []
    for c in range(N_CORES):
        xc = xrep[c * B_LOC : (c + 1) * B_LOC]  # [B_LOC, 64, FLAT]
        xc = np.ascontiguousarray(xc).reshape(PAIRS, 128, FLAT)
        in_maps.append({"xrep": xc, "wts": wts, "bias": biasarr})

    nc = _get_program()
    res = run_bass_kernel_spmd(nc, in_maps, list(range(N_CORES)))
    y = np.concatenate([res.results[c]["y"] for c in range(N_CORES)], axis=0)
    # y is [B, WO, T*2] with layout [b, w', (t, d)]; h' = 2t + d
    y = y.reshape(B, WO, HO).transpose(0, 2, 1)
    return np.ascontiguousarray(y).reshape(B, 1, HO, WO).astype(np.float32)
